# revision 1
# baseline (speedup 1.0000x reference)
"""KoLeo loss (distributed) on 8 Trainium2 NeuronCores.

Strategy: data-parallel over rows. Host normalizes x (the cheap part,
0.05% of FLOPs) and stages the normalized embeddings transposed +
replicated to every core (this is the all-gather, done at input staging).
Each core computes its [1024, 8192] slice of the Gram matrix with a
resident-SBUF bf16 GEMM and extracts the per-row top-8 dot products with
the DVE max instruction directly from PSUM. Because rows are unit-norm,
the self-dot (=1) always ranks first, so no diagonal masking is needed,
and nearest-neighbor distances follow from d^2 = 2 - 2*dot without any
gather. Host reduces the 8x[1024,8] top-8 tables to the scalar loss in
float64.
"""

import sys

sys.path.insert(0, "/opt/trn_rl_repo")

import numpy as np
import ml_dtypes

import concourse.bass as bass
import concourse.tile as tile
from concourse import mybir
from concourse.bass import ds, ts
from concourse.vector_clock import ScopedClock
from concourse.bass_utils import run_bass_kernel_spmd

B = 8192
D = 1024
NCORES = 8
P = 128
MT = (B // NCORES) // P  # 8 row-tiles per core
KC = D // P  # 8 contraction chunks
NW = 4  # column windows of 4 psum banks
WJ = 4  # 512-wide chunks per window
WIN = WJ * 512  # 2048 columns per window

TOPK = 2
GATE_THRESHOLD = 0.5
GATE_ALPHA = 0.1
EPS = 1e-8


class PatchedTileContext(tile.TileContext):
    """The tail drain in this walrus build only tolerates a single sem wait
    per instruction; spill the rest onto standalone wait instructions."""

    def _drain_and_barrier(self, tick_clock, wait_clock):
        nc = self.nc
        drain_inst = nc.sync.drain()
        wait_clock.add_sem_waits(
            drain_inst.ins, ScopedClock({None: tick_clock.global_clock})
        )
        si = drain_inst.ins.sync_info
        if si is not None and len(si.on_wait) > 1:
            waits = list(si.on_wait)
            si.on_wait = waits[:1]
            id2sem = {h.num: h for h in self.sems.allocated().values()}
            for w in waits[1:]:
                nc.sync.wait_ge(id2sem[w.id], w.wait_value)
        nc.all_engine_barrier()
        popped = nc._tile_sem_poison_stack.pop()
        assert popped is self._sem_poison
        nc.clear_and_free_semaphores(list(self.sems.allocated().values()))
        nc.all_engine_barrier()


def _split_excess_waits(nc, max_waits=1):
    """This walrus build rejects instructions carrying more than one sem
    wait; hoist extras onto standalone EventSemaphore instructions placed
    immediately before the over-subscribed instruction on the same engine
    (engines dispatch in order, so this is semantically identical)."""
    for fn in nc.m.functions:
        for bb in fn.blocks:
            insts = bb.instructions
            out = []
            for inst in insts:
                si = inst.sync_info
                if si is not None and len(si.on_wait) > max_waits:
                    waits = list(si.on_wait)
                    for w in waits[:-max_waits]:
                        ev = mybir.InstEventSemaphore(
                            name=nc.get_next_instruction_name(), ins=[], outs=[]
                        )
                        ev.engine = inst.engine
                        ev.sync_info = mybir.SyncInfo(on_wait=[w], on_update=[])
                        out.append(ev)
                    si.on_wait = waits[-max_waits:]
                out.append(inst)
            insts[:] = out


def build_program():
    nc = bass.Bass()
    xt_d = nc.declare_dram_parameter(
        "xt", [KC, P, B], mybir.dt.bfloat16, isOutput=False
    )
    lhsT_d = nc.declare_dram_parameter(
        "lhsT", [KC, P, MT * P], mybir.dt.bfloat16, isOutput=False
    )
    out_d = nc.declare_dram_parameter(
        "top8", [MT, P, 8], mybir.dt.float32, isOutput=True
    )

    with PatchedTileContext(nc) as tc:
        with (
            tc.tile_pool(name="xt_pool", bufs=KC * NW) as xt_pool,
            tc.tile_pool(name="w_pool", bufs=KC) as w_pool,
            tc.tile_pool(name="acc_pool", bufs=1) as acc_pool,
            tc.tile_pool(name="psum", bufs=2, space=bass.MemorySpace.PSUM) as psum_pool,
        ):
            # rhs: full xn.T resident, one tile per (k-chunk, column window)
            # so matmuls only depend on the slice they read.
            xt_sb = [
                [
                    xt_pool.tile([P, WIN], mybir.dt.bfloat16, name="xt_rez")
                    for w in range(NW)
                ]
                for k in range(KC)
            ]
            lhsT_sb = [
                w_pool.tile([P, MT * P], mybir.dt.bfloat16, name="lhsT_rez")
                for _ in range(KC)
            ]
            # interleave weights with window-0 columns so the first matmuls
            # can start as soon as (lhsT_k, xt_k0) pairs land; remaining
            # windows stream in column-major order behind them
            for k in range(KC):
                nc.sync.dma_start(lhsT_sb[k][:], lhsT_d[k])
                for h in range(2):
                    nc.sync.dma_start(
                        xt_sb[k][0][:, ds(h * WIN // 2, WIN // 2)],
                        xt_d[k, :, ds(h * WIN // 2, WIN // 2)],
                    )
            for w in range(1, NW):
                for k in range(KC):
                    nc.sync.dma_start(xt_sb[k][w][:], xt_d[k, :, ds(w * WIN, WIN)])

            # per-(m, w) top-8 staging: [p, m, w, 8]
            t8w = acc_pool.tile([P, MT, NW, 8], mybir.dt.float32)
            out_sb = acc_pool.tile([P, MT, 8], mybir.dt.float32)

            # warm up the PE HAM clock gate during the DMA prologue so the
            # real matmuls run at full clock from the start
            warm_sb = acc_pool.tile([P, 512], mybir.dt.bfloat16)
            nc.gpsimd.memset(warm_sb[:], 0.0)
            warm_ps = psum_pool.tile([P, WJ, 512], mybir.dt.float32, name="psum")
            for i in range(12):
                nc.tensor.matmul(warm_ps[:, i % WJ], warm_sb[:, :P], warm_sb[:])

            for w in range(NW):
                for m in range(MT):
                    psum = psum_pool.tile([P, WJ, 512], mybir.dt.float32)
                    for k in range(KC):
                        lw = lhsT_sb[k][:, ts(m, P)]
                        for j in range(WJ):
                            nc.tensor.matmul(
                                psum[:, j],
                                lw,
                                xt_sb[k][w][:, ts(j, 512)],
                                start=(k == 0),
                                stop=(k == KC - 1),
                            )
                    # top-8 of this 2048-wide window, straight from PSUM
                    nc.vector.max(t8w[:, m, w], psum[:, :, :])
                    if w == NW - 1:
                        # merge this row-tile's window top-8s and store as
                        # soon as its last window is reduced
                        nc.vector.max(out_sb[:, m], t8w[:, m, :, :])
                        nc.sync.dma_start(out_d[m], out_sb[:, m])

    _split_excess_waits(nc)
    return nc


_nc_cache = None


def kernel(x: np.ndarray) -> np.ndarray:
    global _nc_cache
    assert x.shape == (B, D)

    # --- host: normalize (fp64), transpose, shard ---
    x64 = x.astype(np.float64)
    norm = np.sqrt(np.sum(x64 * x64, axis=1, keepdims=True))
    xn = x64 / np.maximum(norm, EPS)
    xt = np.ascontiguousarray(xn.T).astype(ml_dtypes.bfloat16)  # [D, B]
    xt_in = xt.reshape(KC, P, B)

    in_maps = []
    for c in range(NCORES):
        lhsT = np.ascontiguousarray(xt_in[:, :, c * MT * P : (c + 1) * MT * P])
        in_maps.append({"xt": xt_in, "lhsT": lhsT})

    if _nc_cache is None:
        _nc_cache = build_program()
    res = run_bass_kernel_spmd(_nc_cache, in_maps, list(range(NCORES)))

    # --- host: reduce top-8 tables to the scalar loss (fp64) ---
    # top8[c][mt, p, v] -> row c*1024 + mt*128 + p
    tops = np.stack([res.results[c]["top8"] for c in range(NCORES)])  # [NC, MT, P, 8]
    v = tops.reshape(B, 8).astype(np.float64)
    # rank 0 is the self-dot (~1.0); ranks 1..TOPK are the nearest neighbors
    vk = v[:, 1 : 1 + TOPK]  # [B, TOPK]
    d2 = np.maximum(2.0 - 2.0 * vk, 0.0)
    distances = np.sqrt(d2).reshape(-1)
    losses = -np.log(distances + EPS)
    alpha = max(GATE_ALPHA, 1e-6)
    gate = 1.0 / (1.0 + np.exp(-(losses - GATE_THRESHOLD) / alpha))
    lg = losses * gate
    weighted_mean = lg.mean()
    gated_mean = lg.sum() / max(gate.sum(), 1.0)
    out = 0.5 * weighted_mean + 0.5 * gated_mean
    return np.array(out, dtype=np.float32)



# revision 12
# speedup vs baseline: 3.2730x; 3.2730x over previous
"""KoLeo loss (distributed) on 8 Trainium2 NeuronCores.

Strategy: data-parallel over rows. Host normalizes x (fp64), scales by 16
and quantizes to fp8-e4m3 (power-of-2 scale => exact rescale), and stages
the embeddings transposed + column-rotated per core so each core's own
1024 rows sit at columns 0-1023 — the matmul weights then alias the
resident rhs tiles (top-k is column-permutation invariant). Each core
computes its [1024, 8192] Gram slice with fp8 DoubleRow matmuls (2
K-chunks per instruction at 0.5 cycles/row = 4x the bf16 rate). Top-8
extraction per 2048-col window is pipelined across engines: DVE max8
reads one PSUM bank directly; Act copies the other three banks to SBUF
bf16; DVE reduces that half 8:1 with a 3-level tensor_tensor max tree
(packed bf16 SBUF operands hit the 2x DVE perf mode) and max8s the 192
survivors. Top-8 of groupwise maxima preserves the true top-2
neighbors except O(1/B) group collisions; end-to-end host validation of
the full quantization + grouping pipeline gives rel err ~1.8e-3 vs the
2e-2 gate. Host reduces the 8x[1024,8] top-8 tables to the scalar loss
in float64, using d^2 = 2 - 2*dot (rows are unit-norm and the self-dot
ranks first, so no diagonal masking or gather is needed).
"""

import sys

sys.path.insert(0, "/opt/trn_rl_repo")

import numpy as np
import ml_dtypes

import concourse.bass as bass
import concourse.tile as tile
from concourse import mybir
from concourse.bass import ds, ts
from concourse.vector_clock import ScopedClock
from concourse.bass_utils import run_bass_kernel_spmd

B = 8192
D = 1024
NCORES = 8
P = 128
MT = (B // NCORES) // P  # 8 row-tiles per core
KP = D // 256  # 4 DoubleRow contraction pairs (256 dims each)
NW = 4  # column windows
WJ = 4  # 512-wide psum banks per window
WIN = WJ * 512  # 2048 columns per window

SCALE = 16.0  # fp8 pre-scale; power of 2 => exact to undo
TOPK = 2
GATE_THRESHOLD = 0.5
GATE_ALPHA = 0.1
EPS = 1e-8

DR = mybir.MatmulPerfMode.DoubleRow


class PatchedTileContext(tile.TileContext):
    """The tail drain in this walrus build only tolerates a single sem wait
    per instruction; spill the rest onto standalone wait instructions."""

    def _drain_and_barrier(self, tick_clock, wait_clock):
        nc = self.nc
        drain_inst = nc.sync.drain()
        wait_clock.add_sem_waits(
            drain_inst.ins, ScopedClock({None: tick_clock.global_clock})
        )
        si = drain_inst.ins.sync_info
        if si is not None and len(si.on_wait) > 1:
            waits = list(si.on_wait)
            si.on_wait = waits[:1]
            id2sem = {h.num: h for h in self.sems.allocated().values()}
            for w in waits[1:]:
                nc.sync.wait_ge(id2sem[w.id], w.wait_value)
        nc.all_engine_barrier()
        popped = nc._tile_sem_poison_stack.pop()
        assert popped is self._sem_poison
        nc.clear_and_free_semaphores(list(self.sems.allocated().values()))
        nc.all_engine_barrier()


def _split_excess_waits(nc, max_waits=1):
    """This walrus build rejects instructions carrying more than one sem
    wait; hoist extras onto standalone EventSemaphore instructions placed
    immediately before the over-subscribed instruction on the same engine
    (engines dispatch in order, so this is semantically identical)."""
    for fn in nc.m.functions:
        for bb in fn.blocks:
            insts = bb.instructions
            out = []
            for inst in insts:
                si = inst.sync_info
                if si is not None and len(si.on_wait) > max_waits:
                    waits = list(si.on_wait)
                    for w in waits[:-max_waits]:
                        ev = mybir.InstEventSemaphore(
                            name=nc.get_next_instruction_name(), ins=[], outs=[]
                        )
                        ev.engine = inst.engine
                        ev.sync_info = mybir.SyncInfo(on_wait=[w], on_update=[])
                        out.append(ev)
                    si.on_wait = waits[-max_waits:]
                out.append(inst)
            insts[:] = out


def build_program():
    nc = bass.Bass()
    # [P, KP, 2, B]: same dim order as the SBUF tiles — dma_start maps the
    # two sides by flattened linear order, so the orders must agree
    xt_d = nc.declare_dram_parameter(
        "xt8", [P, KP, 2, B], mybir.dt.float8e4, isOutput=False
    )
    out_d = nc.declare_dram_parameter(
        "top8", [MT, P, 8], mybir.dt.float32, isOutput=True
    )

    with PatchedTileContext(nc) as tc:
        with (
            tc.tile_pool(name="xt_pool", bufs=NW) as xt_pool,
            tc.tile_pool(name="cp_pool", bufs=4) as cp_pool,
            tc.tile_pool(name="tr_pool", bufs=3) as tr_pool,
            tc.tile_pool(name="acc_pool", bufs=1) as acc_pool,
            tc.tile_pool(name="psum", bufs=2, space=bass.MemorySpace.PSUM) as psum_pool,
        ):
            # rhs: full fp8 xn.T resident, one tile per column window holding
            # all K-pairs (single big DMA per window — each dma_start holds
            # the HWDGE descriptor generator ~625ns, so fewer is better).
            # Columns are host-rotated so this core's own rows are columns
            # 0-1023: matmul weights alias window-0 slices.
            xt_sb = [
                xt_pool.tile([P, KP, 2, WIN], mybir.dt.float8e4, name="xt_rez")
                for w in range(NW)
            ]
            # window 0 streams in 512-col chunks (matmul dependency
            # granularity); later windows as whole tiles
            for j in range(WJ):
                nc.sync.dma_start(
                    xt_sb[0][:, :, :, ts(j, 512)], xt_d[:, :, :, ts(j, 512)]
                )
            for w in range(1, NW):
                nc.sync.dma_start(xt_sb[w][:], xt_d[:, :, :, ds(w * WIN, WIN)])

            # per-(m, w) top-8 staging: slot 0 = direct psum bank,
            # slot 1 = tree-reduced remainder
            t8 = acc_pool.tile([P, MT, NW, 2, 8], mybir.dt.float32)
            out_sb = acc_pool.tile([P, MT, 8], mybir.dt.float32)

            # warm up the PE HAM clock gate during the DMA prologue so the
            # real matmuls run at full clock from the start; 8 distinct psum
            # banks so the warm matmuls run back-to-back with no WAW syncs
            warm_sb = acc_pool.tile([P, 512], mybir.dt.bfloat16)
            nc.gpsimd.memset(warm_sb[:], 0.0)
            wa = psum_pool.tile([P, 512], mybir.dt.float32, name="ps1")
            wb = psum_pool.tile([P, 3, 512], mybir.dt.float32, name="ps3")
            nc.tensor.matmul(wa[:], warm_sb[:, :P], warm_sb[:])
            nc.tensor.matmul(wb[:, 0], warm_sb[:, :P], warm_sb[:])

            # Each group's 4 banks split across two psum tiles: psA (1 bank)
            # is read only by DVE max8, psB (3 banks) only by the Act copy —
            # decoupled buffer-reuse chains. psA matmuls come first so psA
            # stops early (DVE starts mid-group) and the next-next group's
            # psB writes land after Act's slower release.
            groups = [(w, m) for w in range(NW) for m in range(MT)]
            for gi, (w, m) in enumerate(groups):
                last = gi == len(groups) - 1
                psA = psum_pool.tile([P, 512], mybir.dt.float32, name="ps1")
                psB = psum_pool.tile([P, 3, 512], mybir.dt.float32, name="ps3")
                for c in range(KP):
                    nc.tensor.matmul(
                        psA[:],
                        xt_sb[0][:, c, :, ts(m, P)],
                        xt_sb[w][:, c, :, ts(0, 512)],
                        start=(c == 0),
                        stop=(c == KP - 1),
                        perf_mode=DR,
                    )
                if last:
                    # final group: bank-serial matmuls + split Act copy so
                    # most of the PSUM->SBUF copy overlaps this group's own
                    # matmuls instead of being fully exposed at the tail
                    for j in range(3):
                        for c in range(KP):
                            nc.tensor.matmul(
                                psB[:, j],
                                xt_sb[0][:, c, :, ts(m, P)],
                                xt_sb[w][:, c, :, ds(512 + j * 512, 512)],
                                start=(c == 0),
                                stop=(c == KP - 1),
                                perf_mode=DR,
                            )
                else:
                    for c in range(KP):
                        lw = xt_sb[0][:, c, :, ts(m, P)]
                        for j in range(3):
                            nc.tensor.matmul(
                                psB[:, j],
                                lw,
                                xt_sb[w][:, c, :, ds(512 + j * 512, 512)],
                                start=(c == 0),
                                stop=(c == KP - 1),
                                perf_mode=DR,
                            )
                nc.vector.max(t8[:, m, w, 0], psA[:])
                if True:
                    # Act copies banks 1-3 to packed bf16; DVE reduces 8:1
                    # with a tensor_tensor max tree (2x perf mode), then
                    # max8s the 192 survivors
                    cp = cp_pool.tile([P, 1536], mybir.dt.bfloat16)
                    if last:
                        nc.scalar.copy(cp[:, ds(0, 1024)], psB[:, ds(0, 2), :])
                        nc.scalar.copy(cp[:, ds(1024, 512)], psB[:, 2, :])
                    else:
                        nc.scalar.copy(cp[:], psB[:])
                    r1 = tr_pool.tile([P, 768], mybir.dt.bfloat16, name="r1")
                    nc.vector.tensor_max(r1[:], cp[:, ds(0, 768)], cp[:, ds(768, 768)])
                    r2 = tr_pool.tile([P, 384], mybir.dt.bfloat16, name="r2")
                    nc.vector.tensor_max(r2[:], r1[:, ds(0, 384)], r1[:, ds(384, 384)])
                    r3 = tr_pool.tile([P, 192], mybir.dt.bfloat16, name="r3")
                    nc.vector.tensor_max(r3[:], r2[:, ds(0, 192)], r2[:, ds(192, 192)])
                    nc.vector.max(t8[:, m, w, 1], r3[:])
                if w == NW - 1:
                    # merge this row-tile's window top-8s and store as soon
                    # as its last window is reduced
                    nc.vector.max(out_sb[:, m], t8[:, m])
                    nc.sync.dma_start(out_d[m], out_sb[:, m])

    _split_excess_waits(nc)
    return nc


_nc_cache = None


def kernel(x: np.ndarray) -> np.ndarray:
    global _nc_cache
    assert x.shape == (B, D)

    # --- host: normalize (fp64), scale, quantize fp8, transpose, rotate ---
    x64 = x.astype(np.float64)
    norm = np.sqrt(np.sum(x64 * x64, axis=1, keepdims=True))
    xn = x64 / np.maximum(norm, EPS)
    xq = (SCALE * xn).astype(ml_dtypes.float8_e4m3)  # [B, D]
    # base[p, c, i, n] = xq[n, (2c+i)*128 + p]
    base = np.ascontiguousarray(
        np.ascontiguousarray(xq.T).reshape(KP, 2, P, B).transpose(2, 0, 1, 3)
    )

    in_maps = []
    for c in range(NCORES):
        # rotate so core c's own rows are columns 0-1023 (weights alias)
        arr = np.roll(base, -c * MT * P, axis=3)
        in_maps.append({"xt8": np.ascontiguousarray(arr)})

    if _nc_cache is None:
        _nc_cache = build_program()
    res = run_bass_kernel_spmd(_nc_cache, in_maps, list(range(NCORES)))

    # --- host: reduce top-8 tables to the scalar loss (fp64) ---
    # top8[c][mt, p, v] -> row c*1024 + mt*128 + p
    tops = np.stack([res.results[c]["top8"] for c in range(NCORES)])  # [NC, MT, P, 8]
    v = tops.reshape(B, 8).astype(np.float64) / (SCALE * SCALE)
    # rank 0 is the group containing the self-dot (~1.0); ranks 1..TOPK are
    # the nearest neighbors
    vk = v[:, 1 : 1 + TOPK]  # [B, TOPK]
    d2 = np.maximum(2.0 - 2.0 * vk, 0.0)
    distances = np.sqrt(d2).reshape(-1)
    losses = -np.log(distances + EPS)
    alpha = max(GATE_ALPHA, 1e-6)
    gate = 1.0 / (1.0 + np.exp(-(losses - GATE_THRESHOLD) / alpha))
    lg = losses * gate
    weighted_mean = lg.mean()
    gated_mean = lg.sum() / max(gate.sum(), 1.0)
    out = 0.5 * weighted_mean + 0.5 * gated_mean
    return np.array(out, dtype=np.float32)


# revision 16
# speedup vs baseline: 3.3147x; 1.0127x over previous
"""KoLeo loss (distributed) on 8 Trainium2 NeuronCores.

Strategy: data-parallel over rows. Host normalizes x (fp64), scales by 16
and quantizes to fp8-e4m3 (power-of-2 scale => exact rescale), and stages
the embeddings transposed + column-rotated per core so each core's own
1024 rows sit at columns 0-1023 — the matmul weights then alias the
resident rhs tiles (top-k is column-permutation invariant). Each core
computes its [1024, 8192] Gram slice with fp8 DoubleRow matmuls (2
K-chunks per instruction at 0.5 cycles/row = 4x the bf16 rate). Top-8
extraction per 2048-col window is pipelined across engines: DVE max8
reads one PSUM bank directly; Act copies the other three banks to SBUF
bf16; DVE reduces that half 8:1 with a 3-level tensor_tensor max tree
(packed bf16 SBUF operands hit the 2x DVE perf mode) and max8s the 192
survivors. Top-8 of groupwise maxima preserves the true top-2
neighbors except O(1/B) group collisions; end-to-end host validation of
the full quantization + grouping pipeline gives rel err ~1.8e-3 vs the
2e-2 gate. Host reduces the 8x[1024,8] top-8 tables to the scalar loss
in float64, using d^2 = 2 - 2*dot (rows are unit-norm and the self-dot
ranks first, so no diagonal masking or gather is needed).
"""

import sys

sys.path.insert(0, "/opt/trn_rl_repo")

import numpy as np
import ml_dtypes

import concourse.bass as bass
import concourse.tile as tile
from concourse import mybir
from concourse.bass import ds, ts
from concourse.vector_clock import ScopedClock
from concourse.bass_utils import run_bass_kernel_spmd

B = 8192
D = 1024
NCORES = 8
P = 128
MT = (B // NCORES) // P  # 8 row-tiles per core
KP = D // 256  # 4 DoubleRow contraction pairs (256 dims each)
NW = 4  # column windows
WJ = 4  # 512-wide psum banks per window
WIN = WJ * 512  # 2048 columns per window

SCALE = 16.0  # fp8 pre-scale; power of 2 => exact to undo
TOPK = 2
GATE_THRESHOLD = 0.5
GATE_ALPHA = 0.1
EPS = 1e-8

DR = mybir.MatmulPerfMode.DoubleRow


class PatchedTileContext(tile.TileContext):
    """The tail drain in this walrus build only tolerates a single sem wait
    per instruction; spill the rest onto standalone wait instructions."""

    def _drain_and_barrier(self, tick_clock, wait_clock):
        nc = self.nc
        drain_inst = nc.sync.drain()
        wait_clock.add_sem_waits(
            drain_inst.ins, ScopedClock({None: tick_clock.global_clock})
        )
        si = drain_inst.ins.sync_info
        if si is not None and len(si.on_wait) > 1:
            waits = list(si.on_wait)
            si.on_wait = waits[:1]
            id2sem = {h.num: h for h in self.sems.allocated().values()}
            for w in waits[1:]:
                nc.sync.wait_ge(id2sem[w.id], w.wait_value)
        nc.all_engine_barrier()
        popped = nc._tile_sem_poison_stack.pop()
        assert popped is self._sem_poison
        nc.clear_and_free_semaphores(list(self.sems.allocated().values()))
        nc.all_engine_barrier()


def _split_excess_waits(nc, max_waits=1):
    """This walrus build rejects instructions carrying more than one sem
    wait; hoist extras onto standalone EventSemaphore instructions placed
    immediately before the over-subscribed instruction on the same engine
    (engines dispatch in order, so this is semantically identical)."""
    for fn in nc.m.functions:
        for bb in fn.blocks:
            insts = bb.instructions
            out = []
            for inst in insts:
                si = inst.sync_info
                if si is not None and len(si.on_wait) > max_waits:
                    waits = list(si.on_wait)
                    for w in waits[:-max_waits]:
                        ev = mybir.InstEventSemaphore(
                            name=nc.get_next_instruction_name(), ins=[], outs=[]
                        )
                        ev.engine = inst.engine
                        ev.sync_info = mybir.SyncInfo(on_wait=[w], on_update=[])
                        out.append(ev)
                    si.on_wait = waits[-max_waits:]
                out.append(inst)
            insts[:] = out


def build_program():
    nc = bass.Bass()
    # [P, KP, 2, B]: same dim order as the SBUF tiles — dma_start maps the
    # two sides by flattened linear order, so the orders must agree
    xt_d = nc.declare_dram_parameter(
        "xt8", [P, KP, 2, B], mybir.dt.float8e4, isOutput=False
    )
    out_d = nc.declare_dram_parameter(
        "top8", [MT, P, NW, 2, 8], mybir.dt.float32, isOutput=True
    )

    with PatchedTileContext(nc) as tc:
        with (
            tc.tile_pool(name="xt_pool", bufs=NW) as xt_pool,
            tc.tile_pool(name="cp_pool", bufs=6) as cp_pool,
            tc.tile_pool(name="tr_pool", bufs=4) as tr_pool,
            tc.tile_pool(name="acc_pool", bufs=1) as acc_pool,
            tc.tile_pool(name="psum", bufs=2, space=bass.MemorySpace.PSUM) as psum_pool,
        ):
            # rhs: full fp8 xn.T resident, one tile per column window holding
            # all K-pairs (single big DMA per window — each dma_start holds
            # the HWDGE descriptor generator ~625ns, so fewer is better).
            # Columns are host-rotated so this core's own rows are columns
            # 0-1023: matmul weights alias window-0 slices.
            xt_sb = [
                xt_pool.tile([P, KP, 2, WIN], mybir.dt.float8e4, name="xt_rez")
                for w in range(NW)
            ]
            # window 0 streams in 512-col chunks (matmul dependency
            # granularity); later windows as whole tiles
            for j in range(WJ):
                nc.sync.dma_start(
                    xt_sb[0][:, :, :, ts(j, 512)], xt_d[:, :, :, ts(j, 512)]
                )
            for w in range(1, NW):
                nc.sync.dma_start(xt_sb[w][:], xt_d[:, :, :, ds(w * WIN, WIN)])

            # per-(m, w) top-8 staging: slot 0 = direct psum bank,
            # slot 1 = tree-reduced remainder
            t8 = acc_pool.tile([P, MT, NW, 2, 8], mybir.dt.float32)

            # warm up the PE HAM clock gate during the DMA prologue so the
            # real matmuls run at full clock from the start; 8 distinct psum
            # banks so the warm matmuls run back-to-back with no WAW syncs
            warm_sb = acc_pool.tile([P, 512], mybir.dt.bfloat16)
            nc.gpsimd.memset(warm_sb[:], 0.0)
            wa = psum_pool.tile([P, 512], mybir.dt.float32, name="ps1")
            wb = psum_pool.tile([P, 3, 512], mybir.dt.float32, name="ps3")
            nc.tensor.matmul(wa[:], warm_sb[:, :P], warm_sb[:])
            nc.tensor.matmul(wb[:, 0], warm_sb[:, :P], warm_sb[:])

            # Each group's 4 banks split across two psum tiles: psA (1 bank)
            # is read only by DVE max8, psB (3 banks) only by the Act copy —
            # decoupled buffer-reuse chains. psA matmuls come first so psA
            # stops early (DVE starts mid-group) and the next-next group's
            # psB writes land after Act's slower release.
            groups = [(w, m) for w in range(NW) for m in range(MT)]
            for gi, (w, m) in enumerate(groups):
                last = gi == len(groups) - 1
                psA = psum_pool.tile([P, 512], mybir.dt.float32, name="ps1")
                psB = psum_pool.tile([P, 3, 512], mybir.dt.float32, name="ps3")
                for c in range(KP):
                    nc.tensor.matmul(
                        psA[:],
                        xt_sb[0][:, c, :, ts(m, P)],
                        xt_sb[w][:, c, :, ts(0, 512)],
                        start=(c == 0),
                        stop=(c == KP - 1),
                        perf_mode=DR,
                    )
                if last:
                    # final group: bank-serial matmuls + split Act copy so
                    # most of the PSUM->SBUF copy overlaps this group's own
                    # matmuls instead of being fully exposed at the tail
                    for j in range(3):
                        for c in range(KP):
                            nc.tensor.matmul(
                                psB[:, j],
                                xt_sb[0][:, c, :, ts(m, P)],
                                xt_sb[w][:, c, :, ds(512 + j * 512, 512)],
                                start=(c == 0),
                                stop=(c == KP - 1),
                                perf_mode=DR,
                            )
                else:
                    for c in range(KP):
                        lw = xt_sb[0][:, c, :, ts(m, P)]
                        for j in range(3):
                            nc.tensor.matmul(
                                psB[:, j],
                                lw,
                                xt_sb[w][:, c, :, ds(512 + j * 512, 512)],
                                start=(c == 0),
                                stop=(c == KP - 1),
                                perf_mode=DR,
                            )
                nc.vector.max(t8[:, m, w, 0], psA[:])
                if True:
                    # Act copies banks 1-3 to packed bf16; DVE reduces 8:1
                    # with a tensor_tensor max tree (2x perf mode), then
                    # max8s the 192 survivors
                    cp = cp_pool.tile([P, 1536], mybir.dt.bfloat16)
                    if last:
                        nc.scalar.copy(cp[:, ds(0, 1024)], psB[:, ds(0, 2), :])
                        nc.scalar.copy(cp[:, ds(1024, 512)], psB[:, 2, :])
                    else:
                        nc.scalar.copy(cp[:], psB[:])
                    r1 = tr_pool.tile([P, 768], mybir.dt.bfloat16, name="r1")
                    nc.vector.tensor_max(r1[:], cp[:, ds(0, 768)], cp[:, ds(768, 768)])
                    r2 = tr_pool.tile([P, 384], mybir.dt.bfloat16, name="r2")
                    nc.vector.tensor_max(r2[:], r1[:, ds(0, 384)], r1[:, ds(384, 384)])
                    r3 = tr_pool.tile([P, 192], mybir.dt.bfloat16, name="r3")
                    nc.vector.tensor_max(r3[:], r2[:, ds(0, 192)], r2[:, ds(192, 192)])
                    nc.vector.max(t8[:, m, w, 1], r3[:])
                if w == NW - 1:
                    # ship this row-tile's window top-8 tables as soon as its
                    # last window is reduced; host does the final 64->top3
                    nc.sync.dma_start(out_d[m], t8[:, m])

    _split_excess_waits(nc)
    return nc


_nc_cache = None


def kernel(x: np.ndarray) -> np.ndarray:
    global _nc_cache
    assert x.shape == (B, D)

    # --- host: normalize (fp64), scale, quantize fp8, transpose, rotate ---
    x64 = x.astype(np.float64)
    norm = np.sqrt(np.sum(x64 * x64, axis=1, keepdims=True))
    xn = x64 / np.maximum(norm, EPS)
    xq = (SCALE * xn).astype(ml_dtypes.float8_e4m3)  # [B, D]
    # base[p, c, i, n] = xq[n, (2c+i)*128 + p]
    base = np.ascontiguousarray(
        np.ascontiguousarray(xq.T).reshape(KP, 2, P, B).transpose(2, 0, 1, 3)
    )

    in_maps = []
    for c in range(NCORES):
        # rotate so core c's own rows are columns 0-1023 (weights alias)
        arr = np.roll(base, -c * MT * P, axis=3)
        in_maps.append({"xt8": np.ascontiguousarray(arr)})

    if _nc_cache is None:
        _nc_cache = build_program()
    res = run_bass_kernel_spmd(_nc_cache, in_maps, list(range(NCORES)))

    # --- host: reduce top-8 tables to the scalar loss (fp64) ---
    # top8[c][mt, p, v] -> row c*1024 + mt*128 + p
    tops = np.stack([res.results[c]["top8"] for c in range(NCORES)])  # [NC,MT,P,NW,2,8]
    cand = tops.reshape(B, NW * 2 * 8).astype(np.float64) / (SCALE * SCALE)
    v = -np.sort(-cand, axis=1)[:, : 1 + TOPK]
    vk = v[:, 1 : 1 + TOPK]  # [B, TOPK]
    d2 = np.maximum(2.0 - 2.0 * vk, 0.0)
    distances = np.sqrt(d2).reshape(-1)
    losses = -np.log(distances + EPS)
    alpha = max(GATE_ALPHA, 1e-6)
    gate = 1.0 / (1.0 + np.exp(-(losses - GATE_THRESHOLD) / alpha))
    lg = losses * gate
    weighted_mean = lg.mean()
    gated_mean = lg.sum() / max(gate.sum(), 1.0)
    out = 0.5 * weighted_mean + 0.5 * gated_mean
    return np.array(out, dtype=np.float32)
